# revision 1
# baseline (speedup 1.0000x reference)
"""MLA-style attention (shared latent KV head, attention sink, partial RoPE,
low-rank Q and grouped low-rank output projection) on 8 TRN2 NeuronCores.

Sharding: 64 query heads split 8 per core (tensor parallel on wq_b rows /
wo_a groups); latent KV path replicated; final wo_b matmul computed as
per-core partial products summed on the host.

v5: software-pipelined reps; q-norm folded into softmax exp.  Per rep the body is [stage A (qrT slice via
host-pretransposed xT + seq-sharded kv, all-gather)] -> [stage BC (per-head
q proj, attention, wo_a)] -> [stage E (wo_b, streamed in column quarters)].
Stage A of rep k+1 is traced between BC(k) and E(k) and placed in
right-side SBUF/PSUM pools so it runs concurrently with E(k), hiding the
all-gather.  Constants load once.  The qr RMSnorm is skipped entirely: with
q_norm_w folded into wq_b it is a pure per-row scale that the per-head q
RMSnorm cancels exactly (up to eps noise).  Stage-A loads ride the
Activation-engine DMA queue to stay off the stage-E output queue.
"""

import numpy as np
import ml_dtypes

import concourse.bass as bass
import concourse.mybir as mybir
import concourse.tile as tile
from concourse import bacc
from concourse.bass_utils import run_bass_kernel_spmd
from concourse.masks import make_identity, make_causal_mask

BF16 = mybir.dt.bfloat16
F32 = mybir.dt.float32
AX = mybir.AxisListType
ALU = mybir.AluOpType
ACTF = mybir.ActivationFunctionType

NPBF16 = ml_dtypes.bfloat16

# problem dims (hardcoded; kernel.py must be self-contained)
D, NH, HD, RD, QLR, OLR, OG = 4096, 64, 512, 64, 1024, 1024, 8
S = 1024
NCORES = 8
HPC = NH // NCORES  # query heads per core
EPS = 1e-6
P = 128


class Cfg:
    def __init__(self, s=S, d=D, qlr=QLR, hpc=HPC, olr=OLR, outd=D):
        assert s % P == 0 and d % P == 0 and qlr % 512 == 0 and olr % 512 == 0
        assert outd % 512 == 0
        self.s, self.d, self.qlr, self.hpc, self.olr, self.outd = (
            s, d, qlr, hpc, olr, outd)
        self.sc = s // P        # seq tiles
        self.dc = d // P        # model-dim chunks (contraction for qr/kv)
        self.qc = qlr // P      # q_lora chunks
        self.hc = HD // P       # head-dim chunks (4)
        self.f = hpc * HD       # per-core attention output feature dim
        self.fc = self.f // P   # feature chunks for wo_a contraction
        self.oc = olr // P      # olr chunks (contraction for wo_b)
        self.nc_out = outd // 512  # output D chunks


def _rope_inplace(nc, pool, dst, cos_ap, sin_ap, inverse):
    """Partial RoPE on dst[:, HD-RD:HD] in place. dst is [128, HD] bf16,
    cos/sin are [128, RD//2] f32 for this seq tile."""
    tail = dst[:, HD - RD:HD].rearrange("p (a two) -> p a two", two=2)
    x1 = tail[:, :, 0]
    x2 = tail[:, :, 1]
    t1 = pool.tile([P, RD // 2], F32, tag="rope1")
    t2 = pool.tile([P, RD // 2], F32, tag="rope2")
    t3 = pool.tile([P, RD // 2], F32, tag="rope3")
    t4 = pool.tile([P, RD // 2], F32, tag="rope4")
    nc.vector.tensor_mul(t1[:], x1, cos_ap)
    nc.vector.tensor_mul(t2[:], x2, sin_ap)
    nc.vector.tensor_mul(t3[:], x1, sin_ap)
    nc.vector.tensor_mul(t4[:], x2, cos_ap)
    if not inverse:
        nc.vector.tensor_sub(x1, t1[:], t2[:])
        nc.vector.tensor_add(x2, t3[:], t4[:])
    else:
        nc.vector.tensor_add(x1, t1[:], t2[:])
        nc.vector.tensor_sub(x2, t4[:], t3[:])


def _stage_a(nc, tc, cfg, pc, io):
    """qrT slice + local kv tile + all-gather.  Right-side pools; returns
    (qrt_pool, qrT_sb, kv_sb, kvT_sb); caller releases qrt_pool after BC."""
    sc, dc, qc, hc = cfg.sc, cfg.dc, cfg.qc, cfg.hc
    qrtp = tc.alloc_tile_pool(name="qrt", bufs=1, side="right")
    qrT_sb = qrtp.tile([P, qc, cfg.s], BF16)
    kv_sb = qrtp.tile([P, sc, HD], BF16)
    kvT_sb = qrtp.tile([P, hc, cfg.s], BF16)

    with tc.tile_pool(name="stA", bufs=1, side="right") as pa, \
         tc.tile_pool(name="stAw", bufs=2, side="right") as paw, \
         tc.tile_pool(name="psA", bufs=1, space="PSUM", side="right") as psa:
        wqa_sb = pa.tile([P, dc, P], BF16)
        nc.scalar.dma_start(wqa_sb[:], io["wqa"])

        qr_ps0 = psa.tile([P, 512], F32, tag="qr0", bufs=1)
        qr_ps1 = psa.tile([P, 512], F32, tag="qr1", bufs=1)
        for k in range(dc):
            xT_k = paw.tile([P, cfg.s], BF16, tag="xT", bufs=3)
            nc.scalar.dma_start(xT_k[:], io["xT"][:, k, :])
            st, sp = k == 0, k == dc - 1
            nc.tensor.matmul(qr_ps0[:], wqa_sb[:, k, :],
                             xT_k[:, 0:512], start=st, stop=sp)
            nc.tensor.matmul(qr_ps1[:], wqa_sb[:, k, :],
                             xT_k[:, 512:1024], start=st, stop=sp)
        qrT_loc = paw.tile([P, cfg.s], BF16, tag="qrT_loc", bufs=1)
        nc.any.tensor_copy(qrT_loc[:, 0:512], qr_ps0[:])
        nc.any.tensor_copy(qrT_loc[:, 512:1024], qr_ps1[:])

        # --- kv for this core's seq tile ---
        xt_i = paw.tile([P, dc, P], BF16, tag="xt", bufs=1)
        nc.scalar.dma_start(xt_i[:], io["xt"])
        kv_ps = psa.tile([P, HD], F32, tag="kv", bufs=1)
        gsz = max(1, dc // 4)
        for g in range(dc // gsz):
            wkv_g = paw.tile([P, gsz, HD], BF16, tag="wkv", bufs=2)
            nc.scalar.dma_start(wkv_g[:],
                                io["wkv"][:, g * gsz:(g + 1) * gsz, :])
            for kk in range(gsz):
                k = g * gsz + kk
                nc.tensor.matmul(kv_ps[:], xt_i[:, k, :],
                                 wkv_g[:, kk, :], start=(k == 0),
                                 stop=(k == dc - 1))

        # --- kv epilogue: cast, rmsnorm, weight, rope, transpose ---
        kvt = paw.tile([P, HD], BF16, tag="kvt")
        nc.any.tensor_copy(kvt[:], kv_ps[:])
        sqk = paw.tile([P, HD], BF16, tag="sqk")
        ssqk = paw.tile([P, 1], F32, tag="ssqk")
        nc.scalar.activation(sqk[:], kvt[:], ACTF.Square, accum_out=ssqk[:])
        rtk = paw.tile([P, 1], F32, tag="rtk")
        nc.scalar.activation(rtk[:], ssqk[:], ACTF.Ln,
                             bias=pc["eps"][:, 0:1], scale=1.0 / HD)
        rinvk = paw.tile([P, 1], F32, tag="rinvk")
        nc.scalar.activation(rinvk[:], rtk[:], ACTF.Exp, scale=-0.5)
        kv_dst = paw.tile([P, HD], BF16, tag="kv_loc", bufs=1)
        kv_dst = kv_dst[:]
        nc.scalar.mul(kv_dst, kvt[:], rinvk[:])
        nc.vector.tensor_mul(kv_dst, kv_dst, pc["kvw"][:])
        _rope_inplace(nc, paw, kv_dst, pc["cosm"][:], pc["sinm"][:], False)
        tpk = psa.tile([P, 512], BF16, tag="t", bufs=1)
        for c in range(hc):
            nc.tensor.transpose(tpk[:, c * P:(c + 1) * P],
                                kv_dst[:, c * P:(c + 1) * P], pc["ident"][:])
        kvT_loc = paw.tile([P, hc, P], BF16, tag="kvT_loc", bufs=1)
        nc.any.tensor_copy(kvT_loc[:],
                           tpk[:].rearrange("p (c s) -> p c s", c=hc))

        # pack local results into DRAM and all-gather
        gw = cfg.s + HD + hc * P      # 2048 for full cfg
        with tc.tile_pool(name="ccdram", bufs=1, space="DRAM") as ccd:
            gin = ccd.tile([P, gw], BF16)
            gout = ccd.tile([NCORES, P, gw], BF16, addr_space="Shared")
            nc.scalar.dma_start(gin[:, 0:cfg.s], qrT_loc[:])
            nc.scalar.dma_start(gin[:, cfg.s:cfg.s + HD], kv_dst)
            nc.scalar.dma_start(gin[:, cfg.s + HD:gw],
                                kvT_loc[:].rearrange("p c s -> p (c s)"))
            nc.gpsimd.collective_compute(
                "AllGather", ALU.bypass,
                replica_groups=[list(range(NCORES))],
                ins=[gin[:]], outs=[gout[:]])
            for j in range(NCORES):
                nc.scalar.dma_start(qrT_sb[:, j, :], gout[j, :, 0:cfg.s])
                nc.scalar.dma_start(kv_sb[:, j, :],
                                    gout[j, :, cfg.s:cfg.s + HD])
                nc.scalar.dma_start(kvT_sb[:, :, j * P:(j + 1) * P],
                                    gout[j, :, cfg.s + HD:gw].rearrange(
                                        "p (c s) -> p c s", c=hc))
    return qrtp, qrT_sb, kv_sb, kvT_sb


def _stage_bc(nc, tc, cfg, pc, aa, io, dbg=None):
    """Per-head q proj + attention + wo_a partials into og_acc (f32).
    Returns (og_pool, og_acc); caller releases og_pool after stage E."""
    sc, dc, qc, hc = cfg.sc, cfg.dc, cfg.qc, cfg.hc
    _, qrT_sb, kv_sb, kvT_sb = aa
    ident, cmask, eps_sb = pc["ident"], pc["cmask"], pc["eps"]
    cos_sb, sin_sb = pc["cos"], pc["sin"]
    sink_sb, nsink_sb = pc["sink"], pc["nsink"]
    s_chunks = [(a, min(512, cfg.s - a)) for a in range(0, cfg.s, 512)]

    if dbg:
        nc.sync.dma_start(dbg["qrt"], qrT_sb[:])
        nc.sync.dma_start(dbg["kv"], kv_sb[:])

    ogp = tc.alloc_tile_pool(name="og", bufs=1)
    og_acc = ogp.tile([P, cfg.oc, cfg.s], F32)

    with tc.tile_pool(name="stBCw", bufs=2) as pbw, \
         tc.tile_pool(name="psQ", bufs=1, space="PSUM") as psq, \
         tc.tile_pool(name="psS", bufs=1, space="PSUM") as pss, \
         tc.tile_pool(name="psT", bufs=1, space="PSUM") as pst, \
         tc.tile_pool(name="psO", bufs=1, space="PSUM") as pso, \
         tc.tile_pool(name="psD", bufs=1, space="PSUM") as psd:
        for h in range(cfg.hpc):
            woa_h = pbw.tile([P, hc, cfg.olr], BF16, tag="woa_h", bufs=1)
            nc.sync.dma_start(woa_h[:], io["woa"][:, h * hc:(h + 1) * hc, :])
            wqb_h = pbw.tile([P, qc, HD], BF16, tag="wqb_h", bufs=1)
            nc.sync.dma_start(wqb_h[:], io["wqb"][:, :, h * HD:(h + 1) * HD])
            qT_sb = pbw.tile([P, hc, cfg.s], BF16, tag="qT")
            # ---- q projection + per-head RMS norm + rope ----
            q8 = pbw.tile([P, sc, HD], BF16, tag="q8", bufs=1)
            ssq8 = pbw.tile([P, sc], F32, tag="ssq8")
            for i in range(sc):
                q_ps = psq.tile([P, HD], F32, tag="q", bufs=1)
                for c in range(qc):
                    nc.tensor.matmul(q_ps[:], qrT_sb[:, c, i * P:(i + 1) * P],
                                     wqb_h[:, c, :],
                                     start=(c == 0), stop=(c == qc - 1))
                nc.any.tensor_copy(q8[:, i, :], q_ps[:])
                sqq = pbw.tile([P, HD], BF16, tag="sqq", bufs=1)
                nc.scalar.activation(sqq[:], q8[:, i, :], ACTF.Square,
                                     accum_out=ssq8[:, i:i + 1])
            rt8 = pbw.tile([P, sc], F32, tag="rt8")
            nc.scalar.activation(rt8[:], ssq8[:], ACTF.Ln,
                                 bias=eps_sb[:, 0:1], scale=1.0 / HD)
            rinv8 = pbw.tile([P, sc], F32, tag="rinv8")
            nc.scalar.activation(rinv8[:], rt8[:], ACTF.Exp,
                                 scale=-0.5, bias=eps_sb[:, 1:2])
            for i in range(sc):
                _rope_inplace(nc, pbw, q8[:, i, :],
                              cos_sb[:, i, :], sin_sb[:, i, :], False)
                tpq = pst.tile([P, 512], BF16, tag="t", bufs=2)
                for c in range(hc):
                    nc.tensor.transpose(tpq[:, c * P:(c + 1) * P],
                                        q8[:, i, c * P:(c + 1) * P], ident[:])
                nc.any.tensor_copy(qT_sb[:, :, i * P:(i + 1) * P],
                                   tpq[:].rearrange("p (c s) -> p c s", c=hc))

            if dbg and h == 0:
                nc.sync.dma_start(dbg["qt0"], qT_sb[:])

            # ---- attention for head h ----
            oT_h = pbw.tile([P, hc, cfg.s], BF16, tag="oT_h")
            for i in range(sc):
                w_all = (i + 1) * P
                nch = (w_all + 511) // 512
                s_ps = []
                for ci in range(nch):
                    wci = min(512, w_all - ci * 512)
                    s_ps.append((pss.tile([P, 512], F32, tag="s",
                                          bufs=2, name="s_ps"), wci))
                for k in range(hc):
                    for ci in range(nch):
                        tile_ps, wci = s_ps[ci]
                        nc.tensor.matmul(
                            tile_ps[:, :wci],
                            qT_sb[:, k, i * P:(i + 1) * P],
                            kvT_sb[:, k, ci * 512:ci * 512 + wci],
                            start=(k == 0), stop=(k == hc - 1))
                # causal mask on the diagonal block
                dps, dw = s_ps[-1]
                dcol = (w_all - P) - (nch - 1) * 512
                nc.vector.tensor_add(dps[:, dcol:dcol + P],
                                     dps[:, dcol:dcol + P], cmask[:])
                # negated row max (incl. sink)
                nmt = pbw.tile([P, 3], F32, tag="nmt")
                for ci in range(nch):
                    tile_ps, wci = s_ps[ci]
                    nc.vector.reduce_max(nmt[:, ci:ci + 1], tile_ps[:, :wci],
                                         axis=AX.X, negate=True)
                nm = pbw.tile([P, 1], F32, tag="nm")
                if nch == 1:
                    nc.vector.tensor_mul(nm[:], nmt[:, 0:1],
                                         rinv8[:, i:i + 1])
                else:
                    nc.vector.tensor_tensor(nm[:], nmt[:, 0:1], nmt[:, 1:2],
                                            op=ALU.min)
                    nc.vector.tensor_mul(nm[:], nm[:], rinv8[:, i:i + 1])
                nc.vector.tensor_tensor(nm[:], nm[:],
                                        nsink_sb[:, h:h + 1], op=ALU.min)
                # exp + row sums
                p_sb = pbw.tile([P, cfg.s], BF16, tag="p")
                l0 = pbw.tile([P, 4], F32, tag="l0")
                for ci in range(nch):
                    tile_ps, wci = s_ps[ci]
                    nc.scalar.activation(p_sb[:, ci * 512:ci * 512 + wci],
                                         tile_ps[:, :wci], ACTF.Exp,
                                         bias=nm[:], scale=rinv8[:, i:i + 1],
                                         accum_out=l0[:, ci:ci + 1])
                nc.scalar.activation(l0[:, nch:nch + 1], sink_sb[:, h:h + 1],
                                     ACTF.Exp, bias=nm[:], scale=1.0)
                lsum = pbw.tile([P, 1], F32, tag="lsum")
                nc.vector.reduce_sum(lsum[:], l0[:, :nch + 1], axis=AX.X)
                linv = pbw.tile([P, 1], F32, tag="linv")
                nc.vector.reciprocal(linv[:], lsum[:])
                # transpose p
                pT_sb = pbw.tile([P, cfg.s], BF16, tag="pT")
                for g in range((i + 1 + 3) // 4):
                    jn = min(4, (i + 1) - g * 4)
                    tpp = pst.tile([P, 512], BF16, tag="t", bufs=2)
                    for j4 in range(jn):
                        j = g * 4 + j4
                        nc.tensor.transpose(tpp[:, j4 * P:(j4 + 1) * P],
                                            p_sb[:, j * P:(j + 1) * P],
                                            ident[:])
                    nc.any.tensor_copy(pT_sb[:, g * 512:g * 512 + jn * P],
                                       tpp[:, :jn * P])
                # o = p^T-weighted sum of kv rows
                o_ps = pso.tile([P, HD], F32, tag="o", bufs=1)
                for j in range(i + 1):
                    nc.tensor.matmul(o_ps[:], pT_sb[:, j * P:(j + 1) * P],
                                     kv_sb[:, j, :],
                                     start=(j == 0), stop=(j == i))
                # normalize + inverse rope + store transposed
                o_sb = pbw.tile([P, HD], BF16, tag="o_sb")
                nc.scalar.mul(o_sb[:], o_ps[:], linv[:])
                _rope_inplace(nc, pbw, o_sb[:],
                              cos_sb[:, i, :], sin_sb[:, i, :], True)
                tpo = pst.tile([P, 512], BF16, tag="t", bufs=2)
                for c in range(hc):
                    nc.tensor.transpose(tpo[:, c * P:(c + 1) * P],
                                        o_sb[:, c * P:(c + 1) * P], ident[:])
                nc.any.tensor_copy(oT_h[:, :, i * P:(i + 1) * P],
                                   tpo[:].rearrange("p (c s) -> p c s", c=hc))

            # ---- wo_a partial for this head, into f32 og_acc ----
            for m in range(cfg.oc):
                d_ps = []
                for n2 in range(len(s_chunks)):
                    d_ps.append(psd.tile([P, 512], F32, tag="d",
                                         bufs=2, name="d_ps"))
                for kk in range(hc):
                    for n2, (a, w) in enumerate(s_chunks):
                        nc.tensor.matmul(
                            d_ps[n2][:, :w],
                            woa_h[:, kk, m * P:(m + 1) * P],
                            oT_h[:, kk, a:a + w],
                            start=(kk == 0), stop=(kk == hc - 1))
                for n2, (a, w) in enumerate(s_chunks):
                    if h == 0:
                        nc.vector.tensor_copy(og_acc[:, m, a:a + w],
                                              d_ps[n2][:, :w])
                    else:
                        nc.vector.tensor_add(og_acc[:, m, a:a + w],
                                             og_acc[:, m, a:a + w],
                                             d_ps[n2][:, :w])
    return ogp, og_acc


def _stage_e(nc, tc, cfg, og_acc, io, out_d, dbg=None):
    """out partial = ogT.T @ wob, wob streamed in column quarters."""
    sc = cfg.sc
    s_chunks = [(a, min(512, cfg.s - a)) for a in range(0, cfg.s, 512)]
    ngrp = max(1, cfg.nc_out // 4)   # n-tiles per wob quarter
    with tc.tile_pool(name="stE", bufs=1) as pe, \
         tc.tile_pool(name="stEw", bufs=8, side="right") as pew, \
         tc.tile_pool(name="psE", bufs=1, space="PSUM") as pse:
        ogT_sb = pe.tile([P, cfg.oc, cfg.s], BF16)
        for m in range(cfg.oc):
            for a, w in s_chunks:
                nc.any.tensor_copy(ogT_sb[:, m, a:a + w],
                                   og_acc[:, m, a:a + w])
        if dbg:
            nc.sync.dma_start(dbg["ogt"], ogT_sb[:])
        for nh in range((cfg.nc_out + ngrp - 1) // ngrp):
            ns = [nh * ngrp + j for j in range(ngrp)
                  if nh * ngrp + j < cfg.nc_out]
            wob_h = pe.tile([P, cfg.oc, len(ns) * 512], BF16,
                            tag="wobh", bufs=2)
            for g in range(cfg.oc):
                nc.sync.dma_start(
                    wob_h[:, g:g + 1, :],
                    io["wob"][:, g:g + 1, ns[0] * 512:(ns[-1] + 1) * 512])
            for m in range(sc):
                out_ps = {}
                for n in ns:
                    out_ps[n] = pse.tile([P, 512], F32, tag="out", bufs=4,
                                         name="out_ps")
                for k in range(cfg.oc):
                    for n in ns:
                        nc.tensor.matmul(
                            out_ps[n][:],
                            ogT_sb[:, k, m * P:(m + 1) * P],
                            wob_h[:, k, (n - ns[0]) * 512:
                                  (n - ns[0] + 1) * 512],
                            start=(k == 0), stop=(k == cfg.oc - 1))
                for n in ns:
                    o_out = pew.tile([P, 512], BF16, tag="oo")
                    nc.any.tensor_copy(o_out[:], out_ps[n][:])
                    nc.sync.dma_start(out_d[m, :, n * 512:(n + 1) * 512],
                                      o_out[:])


def build_program(cfg: Cfg, debug=False, reps=1, shard_a=None):
    """Trace + schedule + compile the per-core program. Returns nc."""
    nc = bacc.Bacc("TRN2", debug=False, num_devices=NCORES)

    io = {
        "xt": nc.dram_tensor("xtm", [P, cfg.dc, P], BF16,
                             kind="ExternalInput").ap(),
        "xT": nc.dram_tensor("xT", [P, cfg.dc, cfg.s], BF16,
                             kind="ExternalInput").ap(),
        "wqa": nc.dram_tensor("wqa", [P, cfg.dc, P], BF16,
                              kind="ExternalInput").ap(),
        "wkv": nc.dram_tensor("wkv", [P, cfg.dc, HD], BF16,
                              kind="ExternalInput").ap(),
        "wqb": nc.dram_tensor("wqb", [P, cfg.qc, cfg.hpc * HD], BF16,
                              kind="ExternalInput").ap(),
        "woa": nc.dram_tensor("woa", [P, cfg.fc, cfg.olr], BF16,
                              kind="ExternalInput").ap(),
        "wob": nc.dram_tensor("wob", [P, cfg.oc, cfg.outd], BF16,
                              kind="ExternalInput").ap(),
    }
    cosm_d = nc.dram_tensor("cosm", [P, RD // 2], F32,
                            kind="ExternalInput").ap()
    sinm_d = nc.dram_tensor("sinm", [P, RD // 2], F32,
                            kind="ExternalInput").ap()
    cos_d = nc.dram_tensor("coss", [P, cfg.sc, RD // 2], F32,
                           kind="ExternalInput").ap()
    sin_d = nc.dram_tensor("sins", [P, cfg.sc, RD // 2], F32,
                           kind="ExternalInput").ap()
    kvw_d = nc.dram_tensor("kvw", [P, HD], BF16, kind="ExternalInput").ap()
    sink_d = nc.dram_tensor("sink", [P, cfg.hpc], F32,
                            kind="ExternalInput").ap()
    nsink_d = nc.dram_tensor("nsink", [P, cfg.hpc], F32,
                             kind="ExternalInput").ap()
    out_d = nc.dram_tensor("out", [cfg.sc, P, cfg.outd], BF16,
                           kind="ExternalOutput").ap()
    dbg = {}
    if debug:
        dbg["qrt"] = nc.dram_tensor("dbg_qrt", [P, cfg.qc, cfg.s], BF16,
                                    kind="ExternalOutput").ap()
        dbg["kv"] = nc.dram_tensor("dbg_kv", [P, cfg.sc, HD], BF16,
                                   kind="ExternalOutput").ap()
        dbg["qt0"] = nc.dram_tensor("dbg_qt0", [P, cfg.hc, cfg.s], BF16,
                                    kind="ExternalOutput").ap()
        dbg["ogt"] = nc.dram_tensor("dbg_ogt", [P, cfg.oc, cfg.s], BF16,
                                    kind="ExternalOutput").ap()

    with tile.TileContext(nc) as tc:
        with tc.tile_pool(name="persist", bufs=1) as pp:
            pc = {}
            pc["ident"] = pp.tile([P, P], BF16, name="ident")
            make_identity(nc, pc["ident"][:])
            pc["cmask"] = pp.tile([P, P], F32, name="cmask")
            make_causal_mask(nc, pc["cmask"][:], mask_val=-1e10)
            pc["kvw"] = pp.tile([P, HD], BF16, name="kvw")
            nc.scalar.dma_start(pc["kvw"][:], kvw_d)
            pc["sink"] = pp.tile([P, cfg.hpc], F32, name="sink")
            nc.sync.dma_start(pc["sink"][:], sink_d)
            pc["nsink"] = pp.tile([P, cfg.hpc], F32, name="nsink")
            nc.sync.dma_start(pc["nsink"][:], nsink_d)
            pc["cos"] = pp.tile([P, cfg.sc, RD // 2], F32, name="cos")
            nc.sync.dma_start(pc["cos"][:], cos_d)
            pc["sin"] = pp.tile([P, cfg.sc, RD // 2], F32, name="sin")
            nc.sync.dma_start(pc["sin"][:], sin_d)
            pc["cosm"] = pp.tile([P, RD // 2], F32, name="cosm")
            nc.scalar.dma_start(pc["cosm"][:], cosm_d)
            pc["sinm"] = pp.tile([P, RD // 2], F32, name="sinm")
            nc.scalar.dma_start(pc["sinm"][:], sinm_d)
            pc["eps"] = pp.tile([P, 2], F32, name="eps")
            nc.gpsimd.memset(pc["eps"][:, 0:1], float(EPS))
            nc.gpsimd.memset(pc["eps"][:, 1:2], float(-0.5 * np.log(HD)))

            aa = _stage_a(nc, tc, cfg, pc, io)
            for k in range(reps):
                dbg_k = dbg if (debug and k == 0) else None
                ogp, og_acc = _stage_bc(nc, tc, cfg, pc, aa, io, dbg_k)
                aa[0].release()
                if k + 1 < reps:
                    aa = _stage_a(nc, tc, cfg, pc, io)
                _stage_e(nc, tc, cfg, og_acc, io, out_d, dbg_k)
                ogp.release()

    nc.compile()
    return nc


# ---------------------------------------------------------------------------
# host side
# ---------------------------------------------------------------------------

def _pack_kt(w, n_rows, n_cols):
    """Pack W (given as [n_cols, n_rows] np array) into [128, n_rows/128,
    n_cols] = W.T tiled with the contraction dim on partitions."""
    wt = np.ascontiguousarray(w.T)  # [n_rows, n_cols]
    return np.ascontiguousarray(
        wt.reshape(n_rows // P, P, n_cols).transpose(1, 0, 2))


def prepare_inmaps(inputs, cfg: Cfg):
    bf = NPBF16
    x = np.asarray(inputs["x"], dtype=bf).reshape(cfg.s, cfg.d)
    xt = np.ascontiguousarray(
        x.T.reshape(cfg.dc, P, cfg.sc, P).transpose(2, 1, 0, 3))
    # xT: [P, dc, S] with element [p, k, s] = x[s, k*128+p]
    xT = np.ascontiguousarray(x.T.reshape(cfg.dc, P, cfg.s).transpose(1, 0, 2))

    wq_a = np.asarray(inputs["wq_a"], dtype=bf)
    wqa = _pack_kt(wq_a, cfg.d, cfg.qlr)

    wkv = _pack_kt(np.asarray(inputs["wkv"], dtype=bf), cfg.d, HD)

    q_norm_w = np.asarray(inputs["q_norm_w"], dtype=np.float32)
    wq_b = np.asarray(inputs["wq_b"], dtype=bf).astype(np.float32)
    wq_b = (wq_b * q_norm_w[None, :]).astype(bf)  # fold q_norm into wq_b

    kv_norm_w = np.asarray(inputs["kv_norm_w"], dtype=bf)
    kvw = np.ascontiguousarray(np.broadcast_to(kv_norm_w, (P, HD)))

    cos = np.asarray(inputs["cos"], dtype=np.float32)
    sin = np.asarray(inputs["sin"], dtype=np.float32)
    cos_p = np.ascontiguousarray(
        cos.reshape(cfg.sc, P, RD // 2).transpose(1, 0, 2))
    sin_p = np.ascontiguousarray(
        sin.reshape(cfg.sc, P, RD // 2).transpose(1, 0, 2))

    wo_a = np.asarray(inputs["wo_a"], dtype=bf)  # [OG*OLR, F]
    wo_b = np.asarray(inputs["wo_b"], dtype=bf)  # [D, OG*OLR]
    sink = np.asarray(inputs["attn_sink"], dtype=np.float32)

    xt_tiles = xt  # [sc, P, dc, P]
    in_maps = []
    for c in range(NCORES):
        h0 = c * cfg.hpc
        wqb_c = wq_b[h0 * HD:(h0 + cfg.hpc) * HD, :]  # [hpc*HD, qlr]
        woa_c = wo_a[c * cfg.olr:(c + 1) * cfg.olr, :]  # [olr, F]
        wob_c = wo_b[:, c * cfg.olr:(c + 1) * cfg.olr]  # [outd, olr]
        sink_c = sink[h0:h0 + cfg.hpc]
        in_maps.append({
            "xtm": np.ascontiguousarray(xt_tiles[c]),
            "cosm": np.ascontiguousarray(cos_p[:, c, :]),
            "sinm": np.ascontiguousarray(sin_p[:, c, :]),
            "xT": xT,
            "wqa": np.ascontiguousarray(wqa[:, :, c * P:(c + 1) * P]),
            "wkv": wkv,
            "wqb": _pack_kt(wqb_c, cfg.qlr, cfg.hpc * HD),
            "woa": _pack_kt(woa_c, cfg.f, cfg.olr),
            "wob": _pack_kt(wob_c, cfg.olr, cfg.outd),
            "coss": cos_p,
            "sins": sin_p,
            "kvw": kvw,
            "sink": np.ascontiguousarray(np.broadcast_to(sink_c, (P, cfg.hpc))),
            "nsink": np.ascontiguousarray(
                np.broadcast_to(-sink_c, (P, cfg.hpc))),
        })
    return in_maps


_CACHE = {}


def _get_program():
    if "nc" not in _CACHE:
        _CACHE["nc"] = build_program(Cfg())
    return _CACHE["nc"]


def run(inputs, trace=False):
    """Returns (output [1,S,D] bf16, BassKernelResults)."""
    cfg = Cfg()
    nc = _get_program()
    in_maps = prepare_inmaps(inputs, cfg)
    res = run_bass_kernel_spmd(nc, in_maps, core_ids=list(range(NCORES)),
                               trace=trace)
    acc = np.zeros((cfg.s, cfg.outd), np.float32)
    for r in res.results:
        acc += r["out"].reshape(cfg.s, cfg.outd).astype(np.float32)
    out = acc.astype(NPBF16).reshape(1, cfg.s, cfg.outd)
    return out, res


def kernel(**inputs) -> np.ndarray:
    out, _ = run(inputs)
    return out



# revision 22
# speedup vs baseline: 1.1522x; 1.1522x over previous
"""MLA-style attention (shared latent KV head, attention sink, partial RoPE,
low-rank Q and grouped low-rank output projection) on 8 TRN2 NeuronCores.

Sharding: 64 query heads split 8 per core (tensor parallel on wq_b rows /
wo_a groups); latent KV path replicated; final wo_b matmul computed as
per-core partial products summed on the host.

v5: software-pipelined reps; q-norm folded into softmax exp.  Per rep the body is [stage A (qrT slice via
host-pretransposed xT + seq-sharded kv, all-gather)] -> [stage BC (per-head
q proj, attention, wo_a)] -> [stage E (wo_b, streamed in column quarters)].
Stage A of rep k+1 is traced between BC(k) and E(k) and placed in
right-side SBUF/PSUM pools so it runs concurrently with E(k), hiding the
all-gather.  Constants load once.  The qr RMSnorm is skipped entirely: with
q_norm_w folded into wq_b it is a pure per-row scale that the per-head q
RMSnorm cancels exactly (up to eps noise).  Stage-A loads ride the
Activation-engine DMA queue to stay off the stage-E output queue.
"""

import numpy as np
import ml_dtypes

import concourse.bass as bass
import concourse.mybir as mybir
import concourse.tile as tile
from concourse import bacc
from concourse.bass_utils import run_bass_kernel_spmd
from concourse.masks import make_identity, make_causal_mask

BF16 = mybir.dt.bfloat16
F32 = mybir.dt.float32
AX = mybir.AxisListType
ALU = mybir.AluOpType
ACTF = mybir.ActivationFunctionType

NPBF16 = ml_dtypes.bfloat16

# problem dims (hardcoded; kernel.py must be self-contained)
D, NH, HD, RD, QLR, OLR, OG = 4096, 64, 512, 64, 1024, 1024, 8
S = 1024
NCORES = 8
HPC = NH // NCORES  # query heads per core
EPS = 1e-6
P = 128


class Cfg:
    def __init__(self, s=S, d=D, qlr=QLR, hpc=HPC, olr=OLR, outd=D):
        assert s % P == 0 and d % P == 0 and qlr % 512 == 0 and olr % 512 == 0
        assert outd % 512 == 0
        self.s, self.d, self.qlr, self.hpc, self.olr, self.outd = (
            s, d, qlr, hpc, olr, outd)
        self.sc = s // P        # seq tiles
        self.dc = d // P        # model-dim chunks (contraction for qr/kv)
        self.qc = qlr // P      # q_lora chunks
        self.hc = HD // P       # head-dim chunks (4)
        self.f = hpc * HD       # per-core attention output feature dim
        self.fc = self.f // P   # feature chunks for wo_a contraction
        self.oc = olr // P      # olr chunks (contraction for wo_b)
        self.nc_out = outd // 512  # output D chunks


def _rope_inplace(nc, pool, dst, cos_ap, sin_ap, inverse):
    """Partial RoPE on dst[:, HD-RD:HD] in place. dst is [128, HD] bf16,
    cos/sin are [128, RD//2] f32 for this seq tile."""
    tail = dst[:, HD - RD:HD].rearrange("p (a two) -> p a two", two=2)
    x1 = tail[:, :, 0]
    x2 = tail[:, :, 1]
    t1 = pool.tile([P, RD // 2], F32, tag="rope1", bufs=1)
    t2 = pool.tile([P, RD // 2], F32, tag="rope2", bufs=1)
    t3 = pool.tile([P, RD // 2], F32, tag="rope3", bufs=1)
    t4 = pool.tile([P, RD // 2], F32, tag="rope4", bufs=1)
    nc.vector.tensor_mul(t1[:], x1, cos_ap)
    nc.vector.tensor_mul(t2[:], x2, sin_ap)
    nc.vector.tensor_mul(t3[:], x1, sin_ap)
    nc.vector.tensor_mul(t4[:], x2, cos_ap)
    if not inverse:
        nc.vector.tensor_sub(x1, t1[:], t2[:])
        nc.vector.tensor_add(x2, t3[:], t4[:])
    else:
        nc.vector.tensor_add(x1, t1[:], t2[:])
        nc.vector.tensor_sub(x2, t4[:], t3[:])


NPRE = 2  # xT/wkv groups prefetched during the previous rep's stage E


def _a_prefetch(nc, cfg, io, axtp, groups):
    """Issue xT/wkv group loads into rotating tiles (cross-rep pool)."""
    GK = 4
    out = []
    for g in groups:
        xT_g = axtp.tile([P, GK, cfg.s], BF16, tag="xT", bufs=2)
        nc.scalar.dma_start(xT_g[:], io["xT"][:, g * GK:(g + 1) * GK, :])
        wkv_g = axtp.tile([P, GK, HD], BF16, tag="wkv", bufs=2)
        nc.sync.dma_start(wkv_g[:], io["wkv"][:, g * GK:(g + 1) * GK, :])
        out.append((xT_g, wkv_g))
    return out


def _stage_a(nc, tc, cfg, pc, io, qrtp, axtp, pre):
    """qrT slice + local kv tile + all-gather.  Tiles rotate (bufs=2) in
    the caller-owned qrtp pool so consecutive stage-A instances coexist."""
    sc, dc, qc, hc = cfg.sc, cfg.dc, cfg.qc, cfg.hc
    qrT_sb = qrtp.tile([P, qc, cfg.s], BF16, tag="qrT", bufs=2)
    kv_sb = qrtp.tile([P, sc, HD], BF16, tag="kvg", bufs=2)
    kvT_sb = qrtp.tile([P, hc, cfg.s], BF16, tag="kvT", bufs=1)

    GK = 4  # dc chunk per DMA group
    with tc.tile_pool(name="stAw", bufs=2, side="right") as paw, \
         tc.tile_pool(name="psA", bufs=1, space="PSUM", side="right") as psa:
        wqa_sb = pc["wqa"]
        xt_i = pc["xt"]

        qr_ps0 = psa.tile([P, 512], F32, tag="qr0", bufs=1)
        qr_ps1 = psa.tile([P, 512], F32, tag="qr1", bufs=1)
        kv_ps = psa.tile([P, HD], F32, tag="kv", bufs=1)
        for g in range(dc // GK):
            if g < NPRE:
                xT_g, wkv_g = pre[g]
            else:
                (xT_g, wkv_g), = _a_prefetch(nc, cfg, io, axtp, [g])
            for kk in range(GK):
                k = g * GK + kk
                st, sp = k == 0, k == dc - 1
                nc.tensor.matmul(qr_ps0[:], wqa_sb[:, k, :],
                                 xT_g[:, kk, 0:512], start=st, stop=sp)
                nc.tensor.matmul(qr_ps1[:], wqa_sb[:, k, :],
                                 xT_g[:, kk, 512:1024], start=st, stop=sp)
                nc.tensor.matmul(kv_ps[:], xt_i[:, k, :],
                                 wkv_g[:, kk, :], start=st, stop=sp)
        qrT_loc = paw.tile([P, cfg.s], BF16, tag="qrT_loc", bufs=1)
        nc.any.tensor_copy(qrT_loc[:, 0:512], qr_ps0[:])
        nc.any.tensor_copy(qrT_loc[:, 512:1024], qr_ps1[:])

        # --- kv epilogue: cast, rmsnorm, weight, rope, transpose ---
        kvt = paw.tile([P, HD], BF16, tag="kvt")
        nc.any.tensor_copy(kvt[:], kv_ps[:])
        sqk = paw.tile([P, HD], BF16, tag="sqk")
        ssqk = paw.tile([P, 1], F32, tag="ssqk")
        nc.scalar.activation(sqk[:], kvt[:], ACTF.Square, accum_out=ssqk[:])
        rtk = paw.tile([P, 1], F32, tag="rtk")
        nc.scalar.activation(rtk[:], ssqk[:], ACTF.Ln,
                             bias=pc["eps"][:, 0:1], scale=1.0 / HD)
        rinvk = paw.tile([P, 1], F32, tag="rinvk")
        nc.scalar.activation(rinvk[:], rtk[:], ACTF.Exp, scale=-0.5)
        kv_dst = paw.tile([P, HD], BF16, tag="kv_loc", bufs=1)
        kv_dst = kv_dst[:]
        nc.scalar.mul(kv_dst, kvt[:], rinvk[:])
        nc.vector.tensor_mul(kv_dst, kv_dst, pc["kvw"][:])
        _rope_inplace(nc, paw, kv_dst, pc["cosm"][:], pc["sinm"][:], False)

        # pack local results into DRAM and all-gather (kvT rebuilt locally
        # from the gathered kv at BC start, so it is not shipped)
        gw = cfg.s + HD          # 1536 for full cfg
        with tc.tile_pool(name="ccdram", bufs=1, space="DRAM") as ccd:
            gin = ccd.tile([P, gw], BF16)
            gout = ccd.tile([NCORES, P, gw], BF16, addr_space="Shared")
            nc.scalar.dma_start(gin[:, 0:cfg.s], qrT_loc[:])
            nc.scalar.dma_start(gin[:, cfg.s:cfg.s + HD], kv_dst)
            nc.gpsimd.collective_compute(
                "AllGather", ALU.bypass,
                replica_groups=[list(range(NCORES))],
                ins=[gin[:]], outs=[gout[:]])
            # bulk unpack on the Pool/SWDGE queue (off the HWDGE path)
            gj = gout[:].rearrange("j p w -> p j w")
            nc.gpsimd.dma_start(qrT_sb[:], gj[:, :, 0:cfg.s])
            nc.gpsimd.dma_start(kv_sb[:], gj[:, :, cfg.s:cfg.s + HD])
    return qrT_sb, kv_sb, kvT_sb


def _load_head_weights(nc, cfg, io, wpre, h, eng):
    """Load head h's wo_a / wq_b slices into rotating (bufs=2) tiles."""
    hc, qc = cfg.hc, cfg.qc
    woa_h = wpre.tile([P, hc, cfg.olr], BF16, tag="woa", bufs=2)
    eng.dma_start(woa_h[:], io["woa"][:, h * hc:(h + 1) * hc, :])
    wqb_h = wpre.tile([P, qc, HD], BF16, tag="wqb", bufs=2)
    eng.dma_start(wqb_h[:], io["wqb"][:, :, h * HD:(h + 1) * HD])
    return woa_h, wqb_h


def _woa_head(nc, cfg, pss, woa_h, oT_h, og_acc, first, s_chunks):
    """wo_a partial for one head into og_acc (f32 adds on DVE)."""
    hc = cfg.hc
    for m in range(cfg.oc):
        d_ps = [pss.tile([P, 512], F32, tag="sd", bufs=4, name="d_ps")
                for _ in s_chunks]
        for kk in range(hc):
            for n2, (a, w) in enumerate(s_chunks):
                nc.tensor.matmul(d_ps[n2][:, :w],
                                 woa_h[:, kk, m * P:(m + 1) * P],
                                 oT_h[:, kk, a:a + w],
                                 start=(kk == 0), stop=(kk == hc - 1))
        for n2, (a, w) in enumerate(s_chunks):
            if first:
                nc.vector.tensor_copy(og_acc[:, m, a:a + w], d_ps[n2][:, :w])
            else:
                nc.vector.tensor_add(og_acc[:, m, a:a + w],
                                     og_acc[:, m, a:a + w], d_ps[n2][:, :w])


# triangular packing of pT: row-block j starts at column POFF[j], width S-j*P
POFF = [0]
for _j in range(1, S // P):
    POFF.append(POFF[-1] + S - (_j - 1) * P)
PTW = POFF[-1] + S - (S // P - 1) * P


def _stage_bc(nc, tc, cfg, pc, aa, io, wpre, w_first):
    """v7 BC: per-head q proj; scores computed directly transposed
    (s^T[k,q]), exp without max-subtraction (|s_hat| <= sqrt(HD) bounds it);
    softmax denominator via N=1 matmuls with a ones column; previous head's
    wo_a runs as PE filler under the norm/rope chain."""
    sc, qc, hc = cfg.sc, cfg.qc, cfg.hc
    qrT_sb, kv_sb, kvT_sb = aa
    ident, cmaskT, eps_sb = pc["ident"], pc["cmaskT"], pc["eps"]
    cos_sb, sin_sb = pc["cos"], pc["sin"]
    ones_sb, esink_sb = pc["ones"], pc["esink"]
    s_chunks = [(a, min(512, cfg.s - a)) for a in range(0, cfg.s, 512)]

    ogp = tc.alloc_tile_pool(name="og", bufs=1)
    og_acc = ogp.tile([P, cfg.oc, cfg.s], BF16)

    with tc.tile_pool(name="stBCw", bufs=2) as pbw, \
         tc.tile_pool(name="psQ", bufs=1, space="PSUM") as psq, \
         tc.tile_pool(name="psS", bufs=1, space="PSUM") as pss, \
         tc.tile_pool(name="psT", bufs=1, space="PSUM") as pst, \
         tc.tile_pool(name="psL", bufs=1, space="PSUM") as psl:
        # rebuild kvT from the gathered kv (not shipped in the collective)
        for j in range(sc):
            tpv = pst.tile([P, 512], BF16, tag="t", bufs=2)
            for c in range(hc):
                nc.tensor.transpose(tpv[:, c * P:(c + 1) * P],
                                    kv_sb[:, j, c * P:(c + 1) * P], ident[:])
            nc.scalar.copy(kvT_sb[:, :, j * P:(j + 1) * P],
                           tpv[:].rearrange("p (c s) -> p c s", c=hc))

        prev = None
        w_cur = w_first
        for h in range(cfg.hpc):
            woa_h, wqb_h = w_cur
            if h + 1 < cfg.hpc:
                w_next = _load_head_weights(nc, cfg, io, wpre, h + 1, nc.sync)
            # ---- q projection ----
            q8 = pbw.tile([P, sc, HD], BF16, tag="qo8", bufs=1)
            ssq8 = pbw.tile([P, sc], F32, tag="ssq8", bufs=1)
            for i in range(sc):
                q_ps = psq.tile([P, HD], F32, tag="q", bufs=2)
                for c in range(qc):
                    nc.tensor.matmul(q_ps[:], qrT_sb[:, c, i * P:(i + 1) * P],
                                     wqb_h[:, c, :],
                                     start=(c == 0), stop=(c == qc - 1))
                nc.scalar.copy(q8[:, i, :], q_ps[:])
                sqq = pbw.tile([P, HD], BF16, tag="sqq", bufs=1)
                nc.scalar.activation(sqq[:], q8[:, i, :], ACTF.Square,
                                     accum_out=ssq8[:, i:i + 1])
            # ---- PE filler: previous head's wo_a (covers the norm chain) --
            if prev is not None:
                _woa_head(nc, cfg, pss, prev[1], prev[0], og_acc,
                          first=(h == 1), s_chunks=s_chunks)
            # ---- rms norm scale + rope + transpose ----
            rt8 = pbw.tile([P, sc], F32, tag="rt8", bufs=1)
            nc.scalar.activation(rt8[:], ssq8[:], ACTF.Ln,
                                 bias=eps_sb[:, 0:1], scale=1.0 / HD)
            rinv8 = pbw.tile([P, sc], F32, tag="rinv8", bufs=1)
            nc.scalar.activation(rinv8[:], rt8[:], ACTF.Exp,
                                 scale=-0.5, bias=eps_sb[:, 1:2])
            qT_sb = pbw.tile([P, hc, cfg.s], BF16, tag="qT", bufs=1)
            for i in range(sc):
                nc.scalar.mul(q8[:, i, :], q8[:, i, :], rinv8[:, i:i + 1])
                _rope_inplace(nc, pbw, q8[:, i, :],
                              cos_sb[:, i, :], sin_sb[:, i, :], False)
                tpq = pst.tile([P, 512], BF16, tag="t", bufs=2)
                for c in range(hc):
                    nc.tensor.transpose(tpq[:, c * P:(c + 1) * P],
                                        q8[:, i, c * P:(c + 1) * P], ident[:])
                nc.scalar.copy(qT_sb[:, :, i * P:(i + 1) * P],
                               tpq[:].rearrange("p (c s) -> p c s", c=hc))
            # ---- scores s^T per key tile; exp straight out of PSUM ----
            pT_sb = pbw.tile([P, PTW], BF16, tag="pT", bufs=1)
            for j in range(sc):
                a = j * P
                while a < cfg.s:
                    w = min(512, cfg.s - a)
                    sT_ps = pss.tile([P, 512], F32, tag="sd", bufs=4,
                                     name="sT_ps")
                    for k in range(hc):
                        nc.tensor.matmul(sT_ps[:, :w],
                                         kvT_sb[:, k, j * P:(j + 1) * P],
                                         qT_sb[:, k, a:a + w],
                                         start=(k == 0), stop=(k == hc - 1))
                    if a == j * P:   # causal mask on the diagonal block
                        nc.vector.tensor_add(sT_ps[:, 0:P], sT_ps[:, 0:P],
                                             cmaskT[:])
                    nc.scalar.activation(
                        pT_sb[:, POFF[j] + a - j * P:POFF[j] + a - j * P + w],
                        sT_ps[:, :w], ACTF.Exp)
                    a += w
            # ---- o = p^T-weighted kv; l via N=1 matmuls + sink column ----
            o8 = pbw.tile([P, sc, HD], BF16, tag="qo8", bufs=1)
            l_ps = psl.tile([P, sc], F32, tag="l", bufs=1)
            l_sb = pbw.tile([P, sc], F32, tag="l_sb", bufs=1)
            for i in range(sc):
                o_ps = psq.tile([P, HD], F32, tag="q", bufs=2,
                                name="o_ps")
                for j in range(i + 1):
                    pT_i = pT_sb[:, POFF[j] + (i - j) * P:
                                 POFF[j] + (i - j + 1) * P]
                    nc.tensor.matmul(o_ps[:], pT_i, kv_sb[:, j, :],
                                     start=(j == 0), stop=(j == i))
                for j in range(i + 1):
                    pT_i = pT_sb[:, POFF[j] + (i - j) * P:
                                 POFF[j] + (i - j + 1) * P]
                    nc.tensor.matmul(l_ps[:, i:i + 1], pT_i, ones_sb[:, 0:1],
                                     start=(j == 0), stop=False)
                nc.tensor.matmul(l_ps[:, i:i + 1], esink_sb[0:1, h, :],
                                 ones_sb[0:1, 0:1], start=False, stop=True)
                nc.vector.reciprocal(l_sb[:, i:i + 1], l_ps[:, i:i + 1])
                nc.scalar.mul(o8[:, i, :], o_ps[:], l_sb[:, i:i + 1])
                _rope_inplace(nc, pbw, o8[:, i, :],
                              cos_sb[:, i, :], sin_sb[:, i, :], True)
            oT_h = pbw.tile([P, hc, cfg.s], BF16, tag="oT_h", bufs=2)
            for i in range(sc):
                tpo = pst.tile([P, 512], BF16, tag="t", bufs=2)
                for c in range(hc):
                    nc.tensor.transpose(tpo[:, c * P:(c + 1) * P],
                                        o8[:, i, c * P:(c + 1) * P], ident[:])
                nc.vector.tensor_copy(oT_h[:, :, i * P:(i + 1) * P],
                                       tpo[:].rearrange("p (c s) -> p c s", c=hc))
            prev = (oT_h, woa_h)
            if h + 1 < cfg.hpc:
                w_cur = w_next
        _woa_head(nc, cfg, pss, prev[1], prev[0], og_acc,
                  first=False, s_chunks=s_chunks)
    return ogp, og_acc


NQ = 8  # wob column chunks


def _stage_e_prefetch(nc, tc, cfg, io):
    """Allocate the wob streaming pool and start the first quarter DMAs.
    Traced at rep start so stage E never waits on weight loads."""
    ncq = cfg.nc_out // NQ            # n-tiles (512 cols) per quarter
    wobp = tc.alloc_tile_pool(name="wob", bufs=1)
    tiles = []
    for q in range(NQ):
        t = wobp.tile([P, cfg.oc, ncq * 512], BF16, tag="wobh", bufs=2)
        if q < 2:
            nc.gpsimd.dma_start(
                t[:], io["wob"][:, :, q * ncq * 512:(q + 1) * ncq * 512])
        tiles.append(t)
    return wobp, tiles


def _stage_e(nc, tc, cfg, og_acc, io, out_d, pf):
    """out partial = ogT.T @ wob, wob streamed in column quarters."""
    sc = cfg.sc
    wobp, wob_tiles = pf
    ncq = cfg.nc_out // NQ
    s_chunks = [(a, min(512, cfg.s - a)) for a in range(0, cfg.s, 512)]
    with tc.tile_pool(name="stEw", bufs=2) as pew, \
         tc.tile_pool(name="psE", bufs=1, space="PSUM") as pse:
        ogT_sb = og_acc
        for q in range(NQ):
            wob_h = wob_tiles[q]
            if 0 < q < NQ - 1:
                # prefetch quarter q+1 as soon as quarter q-1's buffer frees
                nc.gpsimd.dma_start(
                    wob_tiles[q + 1][:],
                    io["wob"][:, :, (q + 1) * ncq * 512:(q + 2) * ncq * 512])
            ns = list(range(q * ncq, (q + 1) * ncq))
            for m in range(sc):
                o_row = pew.tile([P, ncq * 512], BF16, tag="orow",
                                 bufs=2, name="o_row")
                out_ps = {}
                for n in ns:
                    out_ps[n] = pse.tile([P, 512], F32, tag="out", bufs=4,
                                         name="out_ps")
                for k in range(cfg.oc):
                    for n in ns:
                        nc.tensor.matmul(
                            out_ps[n][:],
                            ogT_sb[:, k, m * P:(m + 1) * P],
                            wob_h[:, k, (n - ns[0]) * 512:
                                  (n - ns[0] + 1) * 512],
                            start=(k == 0), stop=(k == cfg.oc - 1))
                for n in ns:
                    nc.any.tensor_copy(
                        o_row[:, (n - ns[0]) * 512:(n - ns[0] + 1) * 512],
                        out_ps[n][:])
                nc.gpsimd.dma_start(
                    out_d[m, :, ns[0] * 512:(ns[-1] + 1) * 512], o_row[:])


def build_program(cfg: Cfg, debug=False, reps=1, shard_a=None):
    """Trace + schedule + compile the per-core program. Returns nc."""
    nc = bacc.Bacc("TRN2", debug=False, num_devices=NCORES)

    io = {
        "xt": nc.dram_tensor("xtm", [P, cfg.dc, P], BF16,
                             kind="ExternalInput").ap(),
        "xT": nc.dram_tensor("xT", [P, cfg.dc, cfg.s], BF16,
                             kind="ExternalInput").ap(),
        "wqa": nc.dram_tensor("wqa", [P, cfg.dc, P], BF16,
                              kind="ExternalInput").ap(),
        "wkv": nc.dram_tensor("wkv", [P, cfg.dc, HD], BF16,
                              kind="ExternalInput").ap(),
        "wqb": nc.dram_tensor("wqb", [P, cfg.qc, cfg.hpc * HD], BF16,
                              kind="ExternalInput").ap(),
        "woa": nc.dram_tensor("woa", [P, cfg.fc, cfg.olr], BF16,
                              kind="ExternalInput").ap(),
        "wob": nc.dram_tensor("wob", [P, cfg.oc, cfg.outd], BF16,
                              kind="ExternalInput").ap(),
    }
    cosm_d = nc.dram_tensor("cosm", [P, RD // 2], F32,
                            kind="ExternalInput").ap()
    sinm_d = nc.dram_tensor("sinm", [P, RD // 2], F32,
                            kind="ExternalInput").ap()
    cos_d = nc.dram_tensor("coss", [P, cfg.sc, RD // 2], F32,
                           kind="ExternalInput").ap()
    sin_d = nc.dram_tensor("sins", [P, cfg.sc, RD // 2], F32,
                           kind="ExternalInput").ap()
    kvw_d = nc.dram_tensor("kvw", [P, HD], BF16, kind="ExternalInput").ap()
    esink_d = nc.dram_tensor("esink", [1, cfg.hpc, P], BF16,
                             kind="ExternalInput").ap()
    out_d = nc.dram_tensor("out", [cfg.sc, P, cfg.outd], BF16,
                           kind="ExternalOutput").ap()

    with tile.TileContext(nc) as tc:
        with tc.tile_pool(name="persist", bufs=1) as pp:
            pc = {}
            pc["ident"] = pp.tile([P, P], BF16, name="ident")
            make_identity(nc, pc["ident"][:])
            pc["cmaskT"] = pp.tile([P, P], F32, name="cmaskT")
            # transposed causal mask: 0 where kj <= qi else -1e10
            nc.gpsimd.memset(pc["cmaskT"][:], 0.0)
            nc.gpsimd.affine_select(
                out=pc["cmaskT"][:], in_=pc["cmaskT"][:],
                compare_op=ALU.is_ge, fill=-1e10, base=0,
                pattern=[[1, P]], channel_multiplier=-1)
            pc["ones"] = pp.tile([P, 1], BF16, name="ones")
            nc.gpsimd.memset(pc["ones"][:], 1.0)
            pc["esink"] = pp.tile([1, cfg.hpc, P], BF16, name="esink")
            nc.sync.dma_start(pc["esink"][:], esink_d)
            pc["kvw"] = pp.tile([P, HD], BF16, name="kvw")
            nc.scalar.dma_start(pc["kvw"][:], kvw_d)
            pc["cos"] = pp.tile([P, cfg.sc, RD // 2], F32, name="cos")
            nc.sync.dma_start(pc["cos"][:], cos_d)
            pc["sin"] = pp.tile([P, cfg.sc, RD // 2], F32, name="sin")
            nc.sync.dma_start(pc["sin"][:], sin_d)
            pc["cosm"] = pp.tile([P, RD // 2], F32, name="cosm")
            nc.scalar.dma_start(pc["cosm"][:], cosm_d)
            pc["sinm"] = pp.tile([P, RD // 2], F32, name="sinm")
            nc.scalar.dma_start(pc["sinm"][:], sinm_d)
            pc["eps"] = pp.tile([P, 2], F32, name="eps")
            nc.gpsimd.memset(pc["eps"][:, 0:1], float(EPS))
            nc.gpsimd.memset(pc["eps"][:, 1:2], float(-0.5 * np.log(HD)))
            pc["wqa"] = pp.tile([P, 32, P], BF16, name="wqa_p")
            nc.scalar.dma_start(pc["wqa"][:], io["wqa"])
            pc["xt"] = pp.tile([P, 32, P], BF16, name="xt_p")
            nc.sync.dma_start(pc["xt"][:], io["xt"])


            qrtp = tc.alloc_tile_pool(name="qrt", bufs=1, side="right")
            axtp = tc.alloc_tile_pool(name="axt", bufs=1, side="right")
            wpre = tc.alloc_tile_pool(name="wpre", bufs=1)
            w0 = _load_head_weights(nc, cfg, io, wpre, 0, nc.gpsimd)
            pre = _a_prefetch(nc, cfg, io, axtp, range(NPRE))
            aa = _stage_a(nc, tc, cfg, pc, io, qrtp, axtp, pre)
            for k in range(reps):
                # A(k+1) traced first: its PE work runs before BC(k), so the
                # all-gather launches early and hides under BC(k).
                pf = _stage_e_prefetch(nc, tc, cfg, io)
                aa_next = (_stage_a(nc, tc, cfg, pc, io, qrtp, axtp, pre)
                           if k + 1 < reps else None)
                w0_next = (_load_head_weights(nc, cfg, io, wpre, 0, nc.gpsimd)
                           if k + 1 < reps else None)
                ogp, og_acc = _stage_bc(nc, tc, cfg, pc, aa, io, wpre, w0)
                pre = (_a_prefetch(nc, cfg, io, axtp, range(NPRE))
                       if k + 2 <= reps else None)
                _stage_e(nc, tc, cfg, og_acc, io, out_d, pf)
                ogp.release()
                pf[0].release()
                aa = aa_next
                w0 = w0_next
            wpre.release()
            axtp.release()
            qrtp.release()

    nc.compile()
    return nc


# ---------------------------------------------------------------------------
# host side
# ---------------------------------------------------------------------------

def _pack_kt(w, n_rows, n_cols):
    """Pack W (given as [n_cols, n_rows] np array) into [128, n_rows/128,
    n_cols] = W.T tiled with the contraction dim on partitions."""
    wt = np.ascontiguousarray(w.T)  # [n_rows, n_cols]
    return np.ascontiguousarray(
        wt.reshape(n_rows // P, P, n_cols).transpose(1, 0, 2))


def prepare_inmaps(inputs, cfg: Cfg):
    bf = NPBF16
    x = np.asarray(inputs["x"], dtype=bf).reshape(cfg.s, cfg.d)
    xt = np.ascontiguousarray(
        x.T.reshape(cfg.dc, P, cfg.sc, P).transpose(2, 1, 0, 3))
    # xT: [P, dc, S] with element [p, k, s] = x[s, k*128+p]
    xT = np.ascontiguousarray(x.T.reshape(cfg.dc, P, cfg.s).transpose(1, 0, 2))

    wq_a = np.asarray(inputs["wq_a"], dtype=bf)
    wqa = _pack_kt(wq_a, cfg.d, cfg.qlr)

    wkv = _pack_kt(np.asarray(inputs["wkv"], dtype=bf), cfg.d, HD)

    q_norm_w = np.asarray(inputs["q_norm_w"], dtype=np.float32)
    wq_b = np.asarray(inputs["wq_b"], dtype=bf).astype(np.float32)
    wq_b = (wq_b * q_norm_w[None, :]).astype(bf)  # fold q_norm into wq_b

    kv_norm_w = np.asarray(inputs["kv_norm_w"], dtype=bf)
    kvw = np.ascontiguousarray(np.broadcast_to(kv_norm_w, (P, HD)))

    cos = np.asarray(inputs["cos"], dtype=np.float32)
    sin = np.asarray(inputs["sin"], dtype=np.float32)
    cos_p = np.ascontiguousarray(
        cos.reshape(cfg.sc, P, RD // 2).transpose(1, 0, 2))
    sin_p = np.ascontiguousarray(
        sin.reshape(cfg.sc, P, RD // 2).transpose(1, 0, 2))

    wo_a = np.asarray(inputs["wo_a"], dtype=bf)  # [OG*OLR, F]
    wo_b = np.asarray(inputs["wo_b"], dtype=bf)  # [D, OG*OLR]
    sink = np.asarray(inputs["attn_sink"], dtype=np.float32)

    xt_tiles = xt  # [sc, P, dc, P]
    in_maps = []
    for c in range(NCORES):
        h0 = c * cfg.hpc
        wqb_c = wq_b[h0 * HD:(h0 + cfg.hpc) * HD, :]  # [hpc*HD, qlr]
        woa_c = wo_a[c * cfg.olr:(c + 1) * cfg.olr, :]  # [olr, F]
        wob_c = wo_b[:, c * cfg.olr:(c + 1) * cfg.olr]  # [outd, olr]
        sink_c = sink[h0:h0 + cfg.hpc]
        in_maps.append({
            "xtm": np.ascontiguousarray(xt_tiles[c]),
            "cosm": np.ascontiguousarray(cos_p[:, c, :]),
            "sinm": np.ascontiguousarray(sin_p[:, c, :]),
            "xT": xT,
            "wqa": np.ascontiguousarray(wqa[:, :, c * P:(c + 1) * P]),
            "wkv": wkv,
            "wqb": _pack_kt(wqb_c, cfg.qlr, cfg.hpc * HD),
            "woa": _pack_kt(woa_c, cfg.f, cfg.olr),
            "wob": _pack_kt(wob_c, cfg.olr, cfg.outd),
            "coss": cos_p,
            "sins": sin_p,
            "kvw": kvw,
            "esink": np.ascontiguousarray(np.broadcast_to(
                np.exp(sink_c.astype(np.float32))[None, :, None],
                (1, cfg.hpc, P))).astype(NPBF16),
        })
    return in_maps


_CACHE = {}


def _get_program():
    if "nc" not in _CACHE:
        _CACHE["nc"] = build_program(Cfg())
    return _CACHE["nc"]


def run(inputs, trace=False):
    """Returns (output [1,S,D] bf16, BassKernelResults)."""
    cfg = Cfg()
    nc = _get_program()
    in_maps = prepare_inmaps(inputs, cfg)
    res = run_bass_kernel_spmd(nc, in_maps, core_ids=list(range(NCORES)),
                               trace=trace)
    acc = np.zeros((cfg.s, cfg.outd), np.float32)
    for r in res.results:
        acc += r["out"].reshape(cfg.s, cfg.outd).astype(np.float32)
    out = acc.astype(NPBF16).reshape(1, cfg.s, cfg.outd)
    return out, res


def kernel(**inputs) -> np.ndarray:
    out, _ = run(inputs)
    return out

